# revision 5
# baseline (speedup 1.0000x reference)
"""Trainium2 Bass kernel for nn_Encoder (MHA encoder block).

Problem: x (2, 2048, 1024) fp32; per-head attention (16 heads x 64) with
QKV/O projections + biases; softmax WITHOUT 1/sqrt(hs) scaling.

Sharding (8 cores): core c handles batch n = c//4 and a group of 4 heads
hg = c%4 (features fs = 256*hg .. +256).  Host sums the 4 per-core partial
output projections per batch and adds bo.

v2 layout (16-bit datapath):
  - x^T, Wq, Wk, Wv, q^T, k^T in fp16 (11-bit mantissa; |S| <= ~80 so the
    logit perturbation stays ~1e-3 relative -- exp amplification is safe).
  - E = exp(S), V, ctx, Wo, partial outputs in bf16 (exp(S) reaches ~1e33,
    which overflows fp16; bf16 keeps fp32 range).  PSUM accumulation fp32.
  - fp32r only for the tiny softmax-normalize broadcast path (sel / rr).

Schedule (single core):
  prologue: one resident DMA of x^T (4 tiles) + weights; KT (all keys) and
  QT for query block 0.  Attention runs per 512-query block qb, head pairs
  hp (rows 0-63 / 64-127 run concurrently in the PE array): per key chunk
  kc the S-pair matmuls feed a [128,2,512] PSUM tile, exp (the ACT-bound
  stream, ~60% of runtime) writes bf16 E, and the AV matmuls accumulate
  [ctx~; denom] via the [V | 1] ones-column trick.  All other PE work is
  interleaved INTO the kc loop so it hides under the exp stream: the V
  projection (feature halves, qb 0 only), the next block's QT, and the
  previous block's output projection.  Normalization is fused into the
  PSUM drain: denominators hop partitions via a small DMA, fast-reciprocal
  + a K=4 selector matmul broadcast 1/s to the 128 feature rows, and one
  tensor_tensor multiply writes normalized bf16 ctx.
"""

import numpy as np

HIDDEN = 1024
HEADS = 16
HS = 64
L = 2048
NB = 2
NCORES = 8
HPC = 4          # heads per core
F = HPC * HS     # 256 per-core head features
KC = HIDDEN // 128   # 8 hidden chunks
TB = L // 512        # 4 token blocks of 512
TC = L // 128        # 16 token chunks of 128
KCH = L // 128       # 16 key chunks of 128

_CACHE = {}


def round_fp32r(a: np.ndarray) -> np.ndarray:
    """Round fp32 to the fp32r encoding (12-bit mantissa, round half up)."""
    bits = np.ascontiguousarray(a, dtype=np.float32).view(np.uint32)
    r = ((bits.astype(np.uint64) + 0x800) & 0xFFFFF000).astype(np.uint32)
    return r.view(np.float32)


def _sel_matrix():
    # sel2[j, 64*j : 64*j+64] = 1: broadcasts head-in-pair j's scalar row to
    # its 64 feature partitions
    sel = np.zeros((2, 128), dtype=np.float32)
    for j in range(2):
        sel[j, 64 * j:64 * j + 64] = 1.0
    return sel


def _build(loop_n: int = 1):
    import concourse.mybir as mybir
    import concourse.tile as tile
    from concourse import bacc

    F32 = mybir.dt.float32
    F32R = mybir.dt.float32r
    F16 = mybir.dt.float16
    BF16 = mybir.dt.bfloat16
    AF = mybir.ActivationFunctionType

    nc = bacc.Bacc("TRN2", target_bir_lowering=False, debug=False)

    # wqkv: [p, kc, 3F] = Wq | Wk | Wv feature columns for this core
    xT = nc.dram_tensor("xT", [128, KC, L], F16, kind="ExternalInput")
    wqkv = nc.dram_tensor("wqkv", [128, KC, 3 * F], F16, kind="ExternalInput")
    wo = nc.dram_tensor("wo", [128, 2, HIDDEN], BF16, kind="ExternalInput")
    bqk = nc.dram_tensor("bqk", [128, 2, 2], F32, kind="ExternalInput")
    bv = nc.dram_tensor("bv", [1, F], F16, kind="ExternalInput")
    sel = nc.dram_tensor("sel", [2, 128], F32R, kind="ExternalInput")
    po = nc.dram_tensor("po", [128, TC, HIDDEN], BF16, kind="ExternalOutput")

    with tile.TileContext(nc) as tc:
        with (
            tc.tile_pool(name="const", bufs=1) as const,
            tc.tile_pool(name="xres", bufs=1) as xres,
            tc.tile_pool(name="work", bufs=2) as work,
            tc.tile_pool(name="es", bufs=4) as es,
            tc.tile_pool(name="pout", bufs=3) as pout,
            # PSUM budget (8 banks): s 2x2 + cacc 2x1 + mm 2x1
            tc.tile_pool(name="ps_mm", bufs=2, space="PSUM") as ps_mm,
            tc.tile_pool(name="ps_s", bufs=2, space="PSUM") as ps_s,
            tc.tile_pool(name="ps_c", bufs=2, space="PSUM") as ps_c,
        ):
            # ---------------- persistent tiles + one-time input DMA ----------
            wqkv_sb = const.tile([128, KC, 3 * F], F16)
            nc.sync.dma_start(wqkv_sb, wqkv.ap())
            wo_sb = const.tile([128, 2, HIDDEN], BF16)
            nc.sync.dma_start(wo_sb, wo.ap())
            bqk_sb = const.tile([128, 2, 2], F32)
            nc.sync.dma_start(bqk_sb, bqk.ap())
            bv_sb = const.tile([1, F], F16)
            nc.sync.dma_start(bv_sb, bv.ap())
            sel_r = const.tile([2, 128], F32R)
            nc.sync.dma_start(sel_r, sel.ap())

            wq_sb = wqkv_sb[:, :, 0:F]
            wk_sb = wqkv_sb[:, :, F:2 * F]
            wv_sb = wqkv_sb[:, :, 2 * F:3 * F]

            # resident x^T, one tile per 512-token block
            xt_sb = [
                xres.tile([128, KC, 512], F16, name=f"xt{tb}")
                for tb in range(TB)
            ]

            qt_sb = const.tile([128, 2, L], F16)   # [feat%128, feat//128, q]
            kt_sb = const.tile([128, 2, L], F16)
            # V augmented with a ones column per head: [tok%128, tok//128, h, 65]
            v_sb = const.tile([128, TC, HPC, HS + 1], BF16)
            # normalized ctx^T: [feat%128, feat//128, q]
            c_sb = const.tile([128, 2, L], BF16)
            # softmax denominators [head-in-pair, pair, qb, 512] + f32r recips
            s2_sb = const.tile([2, 2, TB, 512], F32)
            rr_sb = const.tile([2, 2, TB, 512], F32R)

            ones_f = const.tile([1, 128], F32)
            nc.vector.memset(ones_f, 1.0)
            ones_h = const.tile([1, 128], F16)
            nc.vector.tensor_copy(ones_h, ones_f)
            onecol_f = const.tile([128, 1], F32)
            nc.vector.memset(onecol_f, 1.0)
            # ones column of V (col 64 of each head's 65-wide block)
            nc.vector.tensor_copy(
                v_sb[:, :, :, HS:HS + 1],
                onecol_f.to_broadcast((128, TC, HPC, 1)),
            )

            def proj_chain(w_sbuf, b_col, o_sbuf, fc, tb):
                """One [128 feat x 512 tok] projection chain (8 accum MMs)."""
                pt = ps_mm.tile([128, 512], F32, tag="mm")
                for kc in range(KC):
                    nc.tensor.matmul(
                        pt,
                        w_sbuf[:, kc, fc * 128:(fc + 1) * 128],
                        xt_sb[tb][:, kc, :],
                        start=(kc == 0),
                        stop=(kc == KC - 1),
                    )
                nc.vector.tensor_scalar(
                    o_sbuf[:, fc, tb * 512:(tb + 1) * 512],
                    pt,
                    b_col,
                    None,
                    mybir.AluOpType.add,
                )

            def v_half(kc, hp):
                """V projection for key chunk kc, feature half hp (2 heads)."""
                tb, sub = kc // 4, kc % 4
                pv = ps_mm.tile([128, 512], F32, tag="mm")
                for kch in range(KC):
                    nc.tensor.matmul(
                        pv[:, 0:128],
                        xt_sb[tb][:, kch, sub * 128:(sub + 1) * 128],
                        wv_sb[:, kch, hp * 128:(hp + 1) * 128],
                        start=(kch == 0),
                        stop=False,
                    )
                nc.tensor.matmul(
                    pv[:, 0:128], ones_h,
                    bv_sb[:, hp * 128:(hp + 1) * 128],
                    start=False, stop=True,
                )
                nc.vector.tensor_copy(
                    v_sb[:, kc, 2 * hp:2 * hp + 2, 0:HS],
                    pv[:, 0:128].rearrange("p (h s) -> p h s", h=2),
                )

            def po_chain(qb, sub, ot):
                """Output projection for token chunk qb*4+sub, one jb half
                into ot[:, jb*512:...]; caller DMAs the full [128,1024]."""
                t16 = qb * 4 + sub

                def half(jb):
                    pp = ps_mm.tile([128, 512], F32, tag="mm")
                    for chunk in range(2):
                        nc.tensor.matmul(
                            pp,
                            c_sb[:, chunk, t16 * 128:(t16 + 1) * 128],
                            wo_sb[:, chunk, jb * 512:(jb + 1) * 512],
                            start=(chunk == 0),
                            stop=(chunk == 1),
                        )
                    nc.vector.tensor_copy(ot[:, jb * 512:(jb + 1) * 512], pp)

                return half

            def body(_iv=None):
                # resident x^T DMA (per iteration: it is the input stream)
                for tb in range(TB):
                    nc.sync.dma_start(
                        xt_sb[tb], xT.ap()[:, :, tb * 512:(tb + 1) * 512]
                    )
                # ---------- prologue: KT (all blocks) + QT block 0 ----------
                for tb in range(TB):
                    for fc in range(2):
                        proj_chain(wk_sb, bqk_sb[:, fc, 1:2], kt_sb, fc, tb)
                for fc in range(2):
                    proj_chain(wq_sb, bqk_sb[:, fc, 0:1], qt_sb, fc, 0)

                # ---------- attention + fused everything, per query block ---
                for qb in range(TB):
                    for hp in range(2):
                        ha, hb = 2 * hp, 2 * hp + 1
                        cacc_a = ps_c.tile([65, 512], F32, tag="cacc")
                        cacc_b = ps_c.tile([65, 512], F32, tag="cacc")
                        cacc = {ha: cacc_a, hb: cacc_b}
                        # deferred QT for the next query block: one fc chain
                        # burst at the start of each hp pass
                        if qb < TB - 1:
                            proj_chain(
                                wq_sb, bqk_sb[:, hp, 0:1], qt_sb, hp, qb + 1
                            )
                        ets = {}
                        for kc in range(KCH):
                            sp2 = ps_s.tile([128, 2, 512], F32, tag="s")
                            for i, hr in ((0, 0), (1, 64)):
                                nc.tensor.matmul(
                                    sp2[:, i, :],
                                    kt_sb[hr:hr + 64, hp,
                                          kc * 128:(kc + 1) * 128],
                                    qt_sb[hr:hr + 64, hp,
                                          qb * 512:(qb + 1) * 512],
                                    start=True,
                                    stop=True,
                                )
                            et2 = es.tile([128, 2, 512], BF16, tag="e")
                            nc.scalar.activation(et2, sp2, AF.Exp)
                            ets[kc] = et2
                            # interleaved slack work, hidden under the exp
                            # stream: V halves (qb 0) / prev block's PO
                            if qb == 0:
                                v_half(kc, hp)
                            elif hp == 0 and kc % 2 == 0:
                                sub = kc // 4
                                if kc % 4 == 0:
                                    ot = pout.tile([128, HIDDEN], BF16,
                                                   tag="po")
                                    half = po_chain(qb - 1, sub, ot)
                                half(0 if kc % 4 == 0 else 1)
                                if kc % 4 == 2:
                                    t16 = (qb - 1) * 4 + sub
                                    nc.sync.dma_start(po.ap()[:, t16, :], ot)
                            if kc >= 1:
                                prev = ets.pop(kc - 1)
                                for i, h in ((0, ha), (1, hb)):
                                    nc.tensor.matmul(
                                        cacc[h],
                                        v_sb[:, kc - 1, h, :],
                                        prev[:, i, :],
                                        start=(kc - 1 == 0),
                                        stop=False,
                                    )
                        prev = ets.pop(KCH - 1)
                        for i, h in ((0, ha), (1, hb)):
                            nc.tensor.matmul(
                                cacc[h],
                                v_sb[:, KCH - 1, h, :],
                                prev[:, i, :],
                                start=False,
                                stop=True,
                            )
                        # ---- pair drain: denominators -> 1/s -> normalize --
                        for j, h in ((0, ha), (1, hb)):
                            st = work.tile([65, 512], F32, tag="srow")
                            nc.vector.tensor_copy(st[64:65, :],
                                                  cacc[h][64:65, :])
                            nc.sync.dma_start(s2_sb[j:j + 1, hp, qb, :],
                                              st[64:65, :])
                        nc.vector.reciprocal_approx_fast(
                            s2_sb[:, hp, qb, :], s2_sb[:, hp, qb, :]
                        )
                        nc.vector.tensor_copy(rr_sb[:, hp, qb, :],
                                              s2_sb[:, hp, qb, :])
                        bp = ps_mm.tile([128, 512], F32, tag="mm")
                        nc.tensor.matmul(
                            bp, sel_r, rr_sb[:, hp, qb, :],
                            start=True, stop=True,
                        )
                        # DVE can read only one PSUM operand: stage 1/s in SB
                        rb = work.tile([128, 512], F32, tag="rb")
                        nc.vector.tensor_copy(rb, bp)
                        for h, hr in ((ha, 0), (hb, 64)):
                            nc.vector.tensor_tensor(
                                c_sb[hr:hr + 64, hp,
                                     qb * 512:(qb + 1) * 512],
                                cacc[h][0:64, :],
                                rb[hr:hr + 64, :],
                                mybir.AluOpType.mult,
                            )
                # tail: last block's output projection
                for sub in range(4):
                    ot = pout.tile([128, HIDDEN], BF16, tag="po")
                    half = po_chain(TB - 1, sub, ot)
                    half(0)
                    half(1)
                    t16 = (TB - 1) * 4 + sub
                    nc.sync.dma_start(po.ap()[:, t16, :], ot)

            if loop_n > 1:
                with tc.For_i(0, loop_n, 1) as _i:
                    body(_i)
            else:
                body()

    nc.finalize()
    return nc


def _get_nc():
    if "nc" not in _CACHE:
        _CACHE["nc"] = _build()
    return _CACHE["nc"]


def _make_in_maps(x, Wq, bq, Wk, bk, Wv, bv, Wo):
    import ml_dtypes

    BF = ml_dtypes.bfloat16

    # per-batch xT in device layout [p, kc, t]
    xTs = []
    for n in range(NB):
        xt = x[n].T.reshape(KC, 128, L).transpose(1, 0, 2)
        xTs.append(np.ascontiguousarray(xt, dtype=np.float16))

    def wslice(W, fs):
        # [128, KC, F]: [p, kc, f] with hidden = kc*128+p
        return W[fs:fs + F, :].T.reshape(KC, 128, F).transpose(1, 0, 2)

    in_maps = []
    for c in range(NCORES):
        n = c // HPC
        hg = c % HPC
        fs = F * hg
        wqkv = np.concatenate(
            [wslice(Wq, fs), wslice(Wk, fs), wslice(Wv, fs)], axis=2
        ).astype(np.float16)
        wo_d = np.ascontiguousarray(
            Wo[:, fs:fs + F].T.reshape(2, 128, HIDDEN).transpose(1, 0, 2)
        ).astype(BF)
        bqk = np.stack(
            [bq[fs:fs + F].reshape(2, 128).T, bk[fs:fs + F].reshape(2, 128).T],
            axis=2,
        ).astype(np.float32)
        in_maps.append(
            {
                "xT": xTs[n],
                "wqkv": wqkv,
                "wo": wo_d,
                "bqk": np.ascontiguousarray(bqk),
                "bv": bv[fs:fs + F].reshape(1, F).astype(np.float16),
                "sel": round_fp32r(_sel_matrix()),
            }
        )
    return in_maps


def kernel(x, Wq, bq, Wk, bk, Wv, bv, Wo, bo):
    from concourse.bass_utils import run_bass_kernel_spmd

    x = np.asarray(x, dtype=np.float32)
    Wq = np.asarray(Wq, dtype=np.float32)
    Wk = np.asarray(Wk, dtype=np.float32)
    Wv = np.asarray(Wv, dtype=np.float32)
    Wo = np.asarray(Wo, dtype=np.float32)
    bq = np.asarray(bq, dtype=np.float32)
    bk = np.asarray(bk, dtype=np.float32)
    bv = np.asarray(bv, dtype=np.float32)
    bo = np.asarray(bo, dtype=np.float32)

    in_maps = _make_in_maps(x, Wq, bq, Wk, bk, Wv, bv, Wo)
    nc = _get_nc()
    res = run_bass_kernel_spmd(nc, in_maps, core_ids=list(range(NCORES)))

    out = np.zeros((NB, L, HIDDEN), dtype=np.float32)
    for c in range(NCORES):
        n = c // HPC
        p = res.results[c]["po"].astype(np.float32)  # [128, TC, HIDDEN]
        out[n] += p.transpose(1, 0, 2).reshape(L, HIDDEN)
    out += bo
    return out


def _compile_check():
    import tempfile
    from concourse.bass_utils import compile_bass_kernel

    nc = _build()
    td = tempfile.mkdtemp()
    neff = compile_bass_kernel(nc, td)
    print("COMPILE OK:", neff)


if __name__ == "__main__":
    _compile_check()
